# revision 1
# baseline (speedup 1.0000x reference)
"""DGCNN control-points kernel on 8 trn2 NeuronCores.

Sharding: core c -> sample b=c//2, half h=c%2. Each core's x is host-rotated so
its own 1024 points sit in columns 0:1024 (pair AllGather + per-core gather
indices restore each core's frame). Edge conv y = u[.,idx] + v with
u=w[:, :C]@x, v=(w[:,C:]-w[:, :C])@x; BN+lrelu commute with max over k.
kNN: md/2 = G - x2/2 via one fp32 PE matmul chain per 128-row block; top-20 by
3 rounds of DVE max/max_index/match_replace. BN stats: sum_y = sum_n s_n +
K*sum v ; sum_y2 = sum_j cnt_j u_j^2 + 2 sum v*s + K sum v^2; cnt via gpsimd
local_scatter of ones -> M, M as matmul lhsT. Gather: dma_gather of uT rows.
"""
import os
import sys

sys.path.insert(0, "/opt/trn_rl_repo")
import numpy as np
import concourse.bass as bass
import concourse.bacc as bacc
import concourse.tile as tile
from concourse import mybir
from concourse.bass_utils import run_bass_kernel_spmd

AF = mybir.ActivationFunctionType
ALU = mybir.AluOpType
AX = mybir.AxisListType
F32 = mybir.dt.float32
F32R = mybir.dt.float32r
U16 = mybir.dt.uint16
I16 = mybir.dt.int16

B, N, K, NCP = 4, 2048, 20, 400
NH = N // 2
NB = NH // 128
EPS = 1e-5
CS = [3, 64, 128, 256]
OS = [64, 128, 256, 512]
CNT_TOT = float(B * N * K)
DEBUG = bool(os.environ.get("DGCNN_DEBUG"))
STOP1 = int(os.environ.get("DGCNN_STOP", "0"))
SKIPCOLL = int(os.environ.get("DGCNN_SKIPCOLL", "0"))
SKIPGAT = int(os.environ.get("DGCNN_SKIPGAT", "0"))

PAIRS = [[0, 1], [2, 3], [4, 5], [6, 7]]
ALL8 = [list(range(8))]


def _ap(t, dims, offset=0):
    a = t[:] if not isinstance(t, bass.AP) else t
    return bass.AP(tensor=a.tensor, offset=a.offset + offset, ap=list(dims))


def build(debug=False):
    nc = bacc.Bacc("TRN2", target_bir_lowering=False, debug=False, num_devices=8)
    dp = lambda n_, s, d=F32: nc.declare_dram_parameter(n_, s, d, isOutput=False)

    x_in = dp("x", [3, N])
    wdt, wvt, gam, bet, agsel = [], [], [], [], []
    for li in range(4):
        C, O = CS[li], OS[li]
        nt = max(1, O // 128)
        wdt.append(dp(f"wdt{li}", [C, O]))
        wvt.append(dp(f"wvt{li}", [C, O]))
        gam.append(dp(f"g{li}", [128, nt]))
        bet.append(dp(f"b{li}", [128, nt]))
        if li < 3:
            agsel.append(dp(f"agsel{li}", [128, 2 * nt * 8], I16))
    w5t = [dp(f"w5t{j}", [c, 2048]) for j, c in enumerate([3, 64, 128, 256, 512])]
    g5 = dp("g5", [128, 16])
    b5 = dp("b5", [128, 16])
    w6t = dp("w6t", [2048, 256])
    wb6 = dp("wb6", [128, 2])
    g6 = dp("g6", [128, 2])
    b6 = dp("b6", [128, 2])
    w7t = dp("w7t", [4096, 512])
    wb7 = dp("wb7", [128, 4])
    g7 = dp("g7", [128, 4])
    b7 = dp("b7", [128, 4])
    w8t = dp("w8t", [4096, 64])
    wb8 = dp("wb8", [64, 1])
    ident_in = dp("ident", [128, 128])

    y_out = nc.declare_dram_parameter("y_out", [64, B], F32, isOutput=True)
    dbg = {}
    if debug:
        for nm, shp in [("d_x2", [1, N]), ("d_md", [128, N]), ("d_idx", [128, 24]),
                        ("d_x1", [128, N]), ("d_x4", [128, NH]), ("d_p", [128, 16]),
                        ("d_cnt", [128, 16]), ("d_stats", [128, 8]), ("d_gm", [128, 512])]:
            dbg[nm] = nc.declare_dram_parameter(nm, shp, F32, isOutput=True)

    with tile.TileContext(nc) as tc:
        with (
            tc.tile_pool(name="feat", bufs=1) as featp,
            tc.tile_pool(name="work", bufs=2) as workp,
            tc.tile_pool(name="gatp", bufs=1) as gatp,
            tc.tile_pool(name="small", bufs=2) as smallp,
            tc.tile_pool(name="const", bufs=1) as constp,
            tc.tile_pool(name="mdps", bufs=1, space="PSUM") as mdps,
            tc.tile_pool(name="mm", bufs=2, space="PSUM") as mmps,
            tc.tile_pool(name="cntp", bufs=1, space="PSUM") as cntpp,
            tc.tile_pool(name="dram", bufs=1, space="DRAM") as dramp,
        ):
            ident = constp.tile([128, 128], F32)
            nc.sync.dma_start(ident[:], ident_in[:])
            ones_col = constp.tile([128, 1], F32)
            nc.vector.memset(ones_col[:], 1.0)
            x2sb = constp.tile([1, N], F32)
            lhs2 = constp.tile([2, N], F32)
            rhs2 = constp.tile([2, N], F32)
            nc.vector.memset(lhs2[0:1, :], 1.0)
            nc.vector.memset(rhs2[0:2, :], -0.5)
            ones20 = constp.tile([128, 20], mybir.dt.bfloat16)
            nc.vector.memset(ones20[:], 1.0)
            ones_col_bf = constp.tile([128, 1], mybir.dt.bfloat16)
            nc.vector.memset(ones_col_bf[:], 1.0)

            ut_d = dramp.tile([N, 512], F32, tag="utd", name="utd")
            idx_d = dramp.tile([NB * 128, 20], I16, tag="idxd", name="idxd")
            cnt_d = dramp.tile([1, N], F32, tag="cntd", name="cntd")
            agi_d = dramp.tile([256, 1024], F32, tag="agin", name="agin")
            ago_d = dramp.tile([512, 1024], F32, tag="agout", name="agout")
            st_d = dramp.tile([128, 8], F32, tag="stin", name="stin")
            st2_d = dramp.tile([128, 8], F32, tag="stout", name="stout")

            x_t = {0: [featp.tile([128, N], F32, tag="x0", name="x0")]}
            nc.sync.dma_start(x_t[0][0][0:3, :], x_in[:, :])

            def transpose_to(dst_ap, src_ap):
                ps = mmps.tile([128, 512], F32, tag="mm", name="mm")
                nc.tensor.transpose(ps[:, 0:128], src_ap, ident[:])
                nc.scalar.activation(dst_ap, ps[0:src_ap.shape[1], 0:src_ap.shape[0]]
                                     if False else ps[:, 0:128][0:dst_ap.shape[0], 0:dst_ap.shape[1]], AF.Copy)

            for li in range(1 if STOP1 else 4):
                C, O = CS[li], OS[li]
                nct = max(1, C // 128)
                not_ = max(1, O // 128)
                xt = x_t[li]
                crows = [min(128, C - 128 * t) for t in range(nct)]

                # ---- x2 row ----

                for j in range(4):
                    x2ps = mmps.tile([128, 512], F32, tag="mm", name="mm")
                    for t in range(nct):
                        r = crows[t]
                        sqc = workp.tile([128, 512], F32, tag="sqc", name="sqc")
                        nc.vector.tensor_mul(sqc[0:r, :], xt[t][0:r, j * 512:(j + 1) * 512],
                                             xt[t][0:r, j * 512:(j + 1) * 512])
                        nc.tensor.matmul(x2ps[0:1, :], ones_col[0:r, :],
                                         sqc[0:r, :],
                                         start=(t == 0), stop=(t == nct - 1))
                    nc.scalar.activation(x2sb[0:1, j * 512:(j + 1) * 512], x2ps[0:1, :], AF.Copy)
                nc.scalar.activation(rhs2[0:1, :], x2sb[0:1, :], AF.Copy, scale=-0.5)
                nc.sync.dma_start(lhs2[1:2, :], x2sb[0:1, :])
                if debug and li == 0:
                    nc.sync.dma_start(dbg["d_x2"][:], x2sb[0:1, :])

                # ---- v [O-part, NH] ----
                v_t = [featp.tile([128, NH], F32, tag=f"v{t}", name=f"v{t}") for t in range(not_)]
                for ot in range(not_):
                    oc = min(128, O - 128 * ot)
                    wvs = [smallp.tile([128, 128], F32, tag=f"wsl{t}", name=f"wsl{t}", bufs=1) for t in range(nct)]
                    for t in range(nct):
                        r = crows[t]
                        nc.sync.dma_start(wvs[t][0:r, 0:oc], wvt[li][128 * t:128 * t + r, 128 * ot:128 * ot + oc])
                    for j in range(NH // 512):
                        vps = mmps.tile([128, 512], F32, tag="mm", name="mm")
                        for t in range(nct):
                            r = crows[t]
                            nc.tensor.matmul(vps[0:oc, :], wvs[t][0:r, 0:oc],
                                             xt[t][0:r, j * 512:(j + 1) * 512],
                                             start=(t == 0), stop=(t == nct - 1))
                        nc.scalar.activation(v_t[ot][0:oc, j * 512:(j + 1) * 512], vps[0:oc, :], AF.Copy)

                wds_g = []
                for ot in range(not_):
                    oc = min(128, O - 128 * ot)
                    row = [smallp.tile([128, 128], F32, tag=f"wsl2{t}", name=f"wsl2{t}", bufs=1) for t in range(nct)]
                    for t in range(nct):
                        r = crows[t]
                        nc.sync.dma_start(row[t][0:r, 0:oc], wdt[li][128 * t:128 * t + r, 128 * ot:128 * ot + oc])
                    wds_g.append(row)
                # ---- u [O-part, N] in SBUF (gather source + stats) ----
                u_t = [featp.tile([128, N], F32, tag=f"u{t}", name=f"u{t}") for t in range(not_)]
                for ot in range(not_):
                    oc = min(128, O - 128 * ot)
                    for j in range(N // 512):
                        ups = mmps.tile([128, 512], F32, tag="mm", name="mm")
                        for t in range(nct):
                            r = crows[t]
                            nc.tensor.matmul(ups[0:oc, :], wds_g[ot][t][0:r, 0:oc],
                                             xt[t][0:r, j * 512:(j + 1) * 512],
                                             start=(t == 0), stop=(t == nct - 1))
                        nc.scalar.activation(u_t[ot][0:oc, j * 512:(j + 1) * 512], ups[0:oc, :], AF.Copy)

                # ---- per-block: distances, topk, M scatter, idx spill ----
                cntps = cntpp.tile([128, 16], F32, tag="cntp", name="cntp")
                for nb in range(NB):
                    md = mdps.tile([128, N], F32, tag="md", name="md")
                    for j in range(4):
                        js = slice(j * 512, (j + 1) * 512)
                        for t in range(nct):
                            r = crows[t]
                            nc.tensor.matmul(md[:, js], xt[t][0:r, nb * 128:(nb + 1) * 128],
                                             xt[t][0:r, js], start=(t == 0), stop=False)
                        nc.tensor.matmul(md[:, js], lhs2[0:2, nb * 128:(nb + 1) * 128],
                                         rhs2[0:2, js], start=False, stop=True)
                    mds = workp.tile([128, N], F32, tag="mds", name="mds", bufs=1)
                    nc.scalar.activation(mds[:], md[:], AF.Copy)
                    if debug and li == 0 and nb == 0:
                        nc.sync.dma_start(dbg["d_md"][:], mds[:])
                    m8 = smallp.tile([128, 8], F32, tag="m8", name="m8")
                    idx24 = smallp.tile([128, 24], U16, tag="idx24", name="idx24")
                    for rnd in range(3):
                        nc.vector.max(m8[:], mds[:])
                        nc.vector.max_index(idx24[:, rnd * 8:(rnd + 1) * 8], m8[:], mds[:])
                        if rnd < 2:
                            nc.vector.match_replace(mds[:], m8[:], mds[:], -3.0e38)
                    if debug and li == 0 and nb == 0:
                        idxf = smallp.tile([128, 24], F32, tag="idxf", name="idxf")
                        nc.vector.tensor_copy(idxf[:], idx24[:])
                        nc.sync.dma_start(dbg["d_idx"][:], idxf[:])
                    idx20i = smallp.tile([128, 20], I16, tag="idx20i", name="idx20i")
                    nc.vector.tensor_copy(idx20i[:], idx24[:, 0:20])
                    # split scatter: left idx<1024 -> lf, right -> idx-1024 (neg = ignored)
                    idxf = smallp.tile([128, 20], F32, tag="idxf2", name="idxf2")
                    nc.vector.tensor_copy(idxf[:], idx24[:, 0:20])
                    t2 = smallp.tile([128, 20], F32, tag="t2f", name="t2f")
                    nc.vector.tensor_scalar(t2[:], idxf[:], 1023.5, 0.0, ALU.subtract, ALU.max)
                    lff = smallp.tile([128, 20], F32, tag="lff", name="lff")
                    nc.vector.scalar_tensor_tensor(lff[:], t2[:], -4096.0, idxf[:], ALU.mult, ALU.add)
                    lfi = smallp.tile([128, 20], I16, tag="lfi", name="lfi")
                    rfi = smallp.tile([128, 20], I16, tag="rfi", name="rfi")
                    nc.vector.tensor_copy(lfi[:], lff[:])
                    nc.vector.tensor_scalar_sub(rff := smallp.tile([128, 20], F32, tag="rff", name="rff"), idxf[:], 1024.0)
                    nc.vector.tensor_copy(rfi[:], rff[:])
                    M0 = workp.tile([128, NH], mybir.dt.bfloat16, tag="M0", name="M0", bufs=1)
                    M1 = workp.tile([128, NH], mybir.dt.bfloat16, tag="M1", name="M1", bufs=1)
                    nc.gpsimd.local_scatter(M0[:], ones20[:], lfi[:], 128, NH, 20)
                    nc.gpsimd.local_scatter(M1[:], ones20[:], rfi[:], 128, NH, 20)
                    for jc in range(8):
                        nc.tensor.matmul(cntps[:, jc:jc + 1], M0[:, jc * 128:(jc + 1) * 128],
                                         ones_col_bf[:, :], start=(nb == 0), stop=(nb == NB - 1))
                        nc.tensor.matmul(cntps[:, 8 + jc:9 + jc], M1[:, jc * 128:(jc + 1) * 128],
                                         ones_col_bf[:, :], start=(nb == 0), stop=(nb == NB - 1))
                    nc.sync.dma_start(idx_d[nb * 128:(nb + 1) * 128, :], idx20i[:])

                # cnt [128,16] -> DRAM scatter -> cntrep [128, N]
                cnt_sb = smallp.tile([128, 16], F32, tag="cntsb", name="cntsb")
                nc.scalar.activation(cnt_sb[:], cntps[:], AF.Copy)
                if debug and li == 0:
                    nc.sync.dma_start(dbg["d_cnt"][:], cnt_sb[:])
                nc.sync.dma_start(_ap(cnt_d[:], [[1, 128], [128, 16]]), cnt_sb[:])

                # wrapped idx reload
                idxw = featp.tile([128, NB * 160], I16, tag="idxw", name="idxw")
                for g in range(8):
                    src = _ap(idx_d[:], [[1, 16], [2560, NB], [16, 160]])
                    nc.sync.dma_start(idxw[16 * g:16 * (g + 1), :], src)

                # ---- gathers + reduces (ap_gather, [O,(n,k)] layout) ----
                if li < 3:
                    xn = [featp.tile([128, N], F32, tag=f"x{li + 1}_{t}", name=f"x{li + 1}_{t}") for t in range(not_)]
                else:
                    xn = [featp.tile([128, NH], F32, tag=f"x4_{t}", name=f"x4_{t}") for t in range(not_)]
                s1acc = smallp.tile([128, 4], F32, tag="s1acc", name="s1acc", bufs=1)
                cracc = smallp.tile([128, 4], F32, tag="cracc", name="cracc", bufs=1)
                s2acc = smallp.tile([128, 4], F32, tag="s2acc", name="s2acc", bufs=1)
                nc.vector.memset(s1acc[:], 0.0)
                nc.vector.memset(cracc[:], 0.0)
                nc.vector.memset(s2acc[:], 0.0)
                for nb in range(NB):
                    for ot in range(not_):
                        oc = min(128, O - 128 * ot)
                        gat = workp.tile([128, 2560], F32, tag="gat", name="gat")
                        if SKIPGAT:
                            nc.sync.dma_start(gat[0:oc, 0:2048], u_t[ot][0:oc, :])
                        else:
                            nc.gpsimd.ap_gather(gat[0:oc, :], u_t[ot][0:oc, :],
                                                idxw[0:oc, nb * 160:(nb + 1) * 160],
                                                channels=oc if oc >= 16 else 16,
                                                num_elems=N, d=1, num_idxs=2560)
                        gap = bass.AP(tensor=gat[:].tensor, offset=gat[:].offset,
                                      ap=[[list(gat[:].ap[0])[0], oc]] + [[20, 128], [1, 20]])
                        nc.vector.tensor_reduce(xn[ot][0:oc, nb * 128:(nb + 1) * 128], gap, AX.X, ALU.max)
                        sblk = workp.tile([128, 128], F32, tag="sblk", name="sblk")
                        nc.vector.tensor_reduce(sblk[0:oc, :], gap, AX.X, ALU.add)
                        sred = workp.tile([128, 2], F32, tag="sred", name="sred", bufs=1)
                        nc.vector.reduce_sum(sred[0:oc, 0:1], sblk[0:oc, :], AX.X)
                        nc.vector.tensor_add(s1acc[0:oc, ot:ot + 1], s1acc[0:oc, ot:ot + 1], sred[0:oc, 0:1])
                        prodb = workp.tile([128, 128], F32, tag="prodb", name="prodb", bufs=1)
                        nc.vector.tensor_mul(prodb[0:oc, :], sblk[0:oc, :],
                                             v_t[ot][0:oc, nb * 128:(nb + 1) * 128])
                        nc.vector.reduce_sum(sred[0:oc, 1:2], prodb[0:oc, :], AX.X)
                        nc.vector.tensor_add(cracc[0:oc, ot:ot + 1], cracc[0:oc, ot:ot + 1], sred[0:oc, 1:2])
                        sqg = workp.tile([128, 2560], F32, tag="sqg", name="sqg", bufs=1)
                        nc.vector.tensor_mul(sqg[0:oc, :], gat[0:oc, :], gat[0:oc, :])
                        sq2 = workp.tile([128, 2], F32, tag="sq2", name="sq2", bufs=1)
                        nc.vector.reduce_sum(sq2[0:oc, 0:1], sqg[0:oc, :], AX.X)
                        nc.vector.tensor_add(s2acc[0:oc, ot:ot + 1], s2acc[0:oc, ot:ot + 1], sq2[0:oc, 0:1])
                # ---- stats ----
                stat = smallp.tile([128, 8], F32, tag="stat", name="stat")
                scr = smallp.tile([128, 12], F32, tag="scr", name="scr")
                for ot in range(not_):
                    oc = min(128, O - 128 * ot)
                    nc.vector.tensor_add(xn[ot][0:oc, 0:NH], xn[ot][0:oc, 0:NH], v_t[ot][0:oc, :])
                    nc.vector.tensor_copy(scr[0:oc, 4:5], s2acc[0:oc, ot:ot + 1])  # S2a
                    nc.vector.tensor_copy(scr[0:oc, 6:7], s1acc[0:oc, ot:ot + 1])          # S1a
                    nc.vector.reduce_sum(scr[0:oc, 7:8], v_t[ot][0:oc, :], AX.X)          # Sv
                    tmp = workp.tile([128, NH], F32, tag="stmp", name="stmp", bufs=1)
                    nc.vector.tensor_mul(tmp[0:oc, :], v_t[ot][0:oc, :], v_t[ot][0:oc, :])
                    nc.vector.reduce_sum(scr[0:oc, 8:9], tmp[0:oc, :], AX.X)              # Sv2
                    nc.vector.tensor_copy(scr[0:oc, 9:10], cracc[0:oc, ot:ot + 1])         # cross
                    nc.vector.tensor_scalar_mul(scr[0:oc, 10:11], scr[0:oc, 7:8], float(K))
                    nc.vector.tensor_add(stat[0:oc, 2 * ot:2 * ot + 1], scr[0:oc, 6:7], scr[0:oc, 10:11])
                    nc.vector.tensor_scalar_mul(scr[0:oc, 10:11], scr[0:oc, 8:9], float(K))
                    nc.vector.tensor_scalar_mul(scr[0:oc, 11:12], scr[0:oc, 9:10], 2.0)
                    nc.vector.tensor_add(scr[0:oc, 10:11], scr[0:oc, 10:11], scr[0:oc, 11:12])
                    nc.vector.tensor_add(stat[0:oc, 2 * ot + 1:2 * ot + 2], scr[0:oc, 4:5], scr[0:oc, 10:11])

                nc.sync.dma_start(st_d[:, :], stat[:, :])
                if SKIPCOLL:
                    nc.gpsimd.dma_start(st2_d[:, :], st_d[:, :])
                else:
                    nc.gpsimd.collective_compute("AllReduce", ALU.add, replica_groups=ALL8,
                                                 ins=[st_d[:].opt()], outs=[st2_d[:].opt()])
                statg = smallp.tile([128, 8], F32, tag="statg", name="statg")
                nc.sync.dma_start(statg[:], st2_d[:, :])
                if debug and li == 0:
                    nc.sync.dma_start(dbg["d_stats"][:], statg[:])

                # bn scale/bias per O-tile
                gpt = smallp.tile([128, 4], F32, tag="gpt", name="gpt")
                bpt = smallp.tile([128, 4], F32, tag="bpt", name="bpt")
                nc.sync.dma_start(gpt[:, 0:not_], gam[li][:, :])
                nc.sync.dma_start(bpt[:, 0:not_], bet[li][:, :])
                mean = smallp.tile([128, 4], F32, tag="mean", name="mean")
                var = smallp.tile([128, 4], F32, tag="var", name="var")
                scl = smallp.tile([128, 4], F32, tag="scl", name="scl")
                bia = smallp.tile([128, 4], F32, tag="bia", name="bia")
                for ot in range(not_):
                    nc.vector.tensor_scalar_mul(mean[:, ot:ot + 1], statg[:, 2 * ot:2 * ot + 1], 1.0 / CNT_TOT)
                    nc.vector.tensor_scalar_mul(var[:, ot:ot + 1], statg[:, 2 * ot + 1:2 * ot + 2], 1.0 / CNT_TOT)
                nc.vector.tensor_mul(scl[:, 0:not_], mean[:, 0:not_], mean[:, 0:not_])
                nc.vector.tensor_sub(var[:, 0:not_], var[:, 0:not_], scl[:, 0:not_])
                nc.vector.tensor_scalar_add(var[:, 0:not_], var[:, 0:not_], EPS)
                nc.scalar.activation(var[:, 0:not_], var[:, 0:not_], AF.Sqrt)
                nc.vector.reciprocal(var[:, 0:not_], var[:, 0:not_])
                nc.vector.tensor_mul(scl[:, 0:not_], gpt[:, 0:not_], var[:, 0:not_])
                nc.vector.tensor_mul(bia[:, 0:not_], mean[:, 0:not_], scl[:, 0:not_])
                nc.vector.tensor_sub(bia[:, 0:not_], bpt[:, 0:not_], bia[:, 0:not_])

                if li < 3:
                    for ot in range(not_):
                        nc.sync.dma_start(agi_d[ot * 128:(ot + 1) * 128, :], xn[ot][:, 0:NH])
                    if SKIPCOLL:
                        nc.gpsimd.dma_start(ago_d[0:not_ * 128, :], agi_d[0:not_ * 128, :])
                    else:
                        nc.gpsimd.collective_compute(
                            "AllReduce", ALU.add, replica_groups=PAIRS,
                            ins=[agi_d[0:not_ * 128, :].opt()], outs=[ago_d[0:not_ * 128, :].opt()])
                    for ot in range(not_):
                        nc.sync.dma_start(xn[ot][:, NH:N], ago_d[ot * 128:(ot + 1) * 128, :])
                        nc.vector.tensor_sub(xn[ot][:, NH:N], xn[ot][:, NH:N], xn[ot][:, 0:NH])
                        nc.scalar.activation(xn[ot][:, :], xn[ot][:, :], AF.Identity,
                                             scale=scl[:, ot:ot + 1], bias=bia[:, ot:ot + 1])
                        nc.vector.scalar_tensor_tensor(xn[ot][:, :], xn[ot][:, :], 0.2,
                                                       xn[ot][:, :], ALU.mult, ALU.max)
                    x_t[li + 1] = xn
                else:
                    for ot in range(not_):
                        nc.scalar.activation(xn[ot][:, :], xn[ot][:, :], AF.Identity,
                                             scale=scl[:, ot:ot + 1], bias=bia[:, ot:ot + 1])
                        nc.vector.scalar_tensor_tensor(xn[ot][:, :], xn[ot][:, :], 0.2,
                                                       xn[ot][:, :], ALU.mult, ALU.max)
                    x_t[4] = xn
                if debug and li == 0:
                    nc.sync.dma_start(dbg["d_x1"][:], x_t[1][0][:, :])
                if debug and li == 3:
                    nc.sync.dma_start(dbg["d_x4"][:], x_t[4][0][:, :])

            if STOP1:
                y8s = smallp.tile([64, 4], F32, tag="y8s", name="y8s")
                nc.vector.tensor_copy(y8s[:], x_t[1][0][0:64, 0:4])
                nc.sync.dma_start(y_out[:, :], y8s[:])
            if not STOP1:
                # ---------------- L5 + pool ----------------
                hsrc = [(x_t[0][0], 3, w5t[0], 0), (x_t[1][0], 64, w5t[1], 0),
                        (x_t[2][0], 128, w5t[2], 0), (x_t[3][0], 128, w5t[3], 0),
                        (x_t[3][1], 128, w5t[3], 128)]
                for t in range(4):
                    hsrc.append((x_t[4][t], 128, w5t[4], 128 * t))
                s1t = smallp.tile([128, 16], F32, tag="s1t", name="s1t")
                s2t = smallp.tile([128, 16], F32, tag="s2t", name="s2t")
                pmx = smallp.tile([128, 16], F32, tag="pmx", name="pmx")
                acc1 = smallp.tile([128, 2], F32, tag="acc1", name="acc1")
                for ob in range(16):
                    h5s = workp.tile([128, NH], F32, tag="h5s", name="h5s", bufs=1)
                    for jj in range(2):
                        h5ps = mmps.tile([128, 512], F32, tag="mm", name="mm")
                        js = slice(jj * 512, jj * 512 + 512)
                        for si, (xt_, r, wp, roff) in enumerate(hsrc):
                            ws = smallp.tile([128, 128], F32, tag="w5s", name="w5s")
                            nc.sync.dma_start(ws[0:r, :], wp[roff:roff + r, ob * 128:(ob + 1) * 128])
                            nc.tensor.matmul(h5ps[:, :], ws[0:r, :], xt_[0:r, js],
                                             start=(si == 0), stop=(si == len(hsrc) - 1))
                        nc.scalar.activation(h5s[:, js], h5ps[:, :], AF.Copy, accum_out=acc1[:, jj:jj + 1])
                    nc.vector.tensor_add(s1t[:, ob:ob + 1], acc1[:, 0:1], acc1[:, 1:2])
                    sq5 = workp.tile([128, NH], F32, tag="sq5", name="sq5", bufs=1)
                    nc.scalar.activation(sq5[:, :], h5s[:, :], AF.Square, accum_out=s2t[:, ob:ob + 1])
                    nc.vector.reduce_max(pmx[:, ob:ob + 1], h5s[:, :], AX.X)

                s12 = smallp.tile([128, 32], F32, tag="s12", name="s12")
                nc.vector.tensor_copy(s12[:, 0:16], s1t[:, :])
                nc.vector.tensor_copy(s12[:, 16:32], s2t[:, :])
                st32_d = dramp.tile([128, 32], F32, tag="st32", name="st32")
                st32o_d = dramp.tile([128, 32], F32, tag="st32o", name="st32o")
                nc.sync.dma_start(st32_d[:, :], s12[:, :])
                nc.gpsimd.collective_compute("AllReduce", ALU.add, replica_groups=ALL8,
                                             ins=[st32_d[:].opt()], outs=[st32o_d[:].opt()])
                nc.sync.dma_start(s12[:, :], st32o_d[:, :])
                pm_d = dramp.tile([128, 16], F32, tag="pmd", name="pmd")
                pm2_d = dramp.tile([128, 16], F32, tag="pmd2", name="pmd2")
                nc.sync.dma_start(pm_d[:, :], pmx[:, :])
                nc.gpsimd.collective_compute("AllReduce", ALU.max, replica_groups=PAIRS,
                                             ins=[pm_d[:].opt()], outs=[pm2_d[:].opt()])
                nc.sync.dma_start(pmx[:, :], pm2_d[:, :])

                g5t = smallp.tile([128, 16], F32, tag="g5t", name="g5t")
                b5t = smallp.tile([128, 16], F32, tag="b5t", name="b5t")
                nc.sync.dma_start(g5t[:], g5[:, :])
                nc.sync.dma_start(b5t[:], b5[:, :])
                m5 = smallp.tile([128, 16], F32, tag="m5", name="m5")
                v5 = smallp.tile([128, 16], F32, tag="v5", name="v5")
                t5 = smallp.tile([128, 16], F32, tag="t5", name="t5")
                p5 = smallp.tile([128, 16], F32, tag="p5", name="p5")
                nc.vector.tensor_scalar_mul(m5[:], s12[:, 0:16], 1.0 / (B * N))
                nc.vector.tensor_scalar_mul(v5[:], s12[:, 16:32], 1.0 / (B * N))
                nc.vector.tensor_mul(t5[:], m5[:], m5[:])
                nc.vector.tensor_sub(v5[:], v5[:], t5[:])
                nc.vector.tensor_scalar_add(v5[:], v5[:], EPS)
                nc.scalar.activation(v5[:], v5[:], AF.Sqrt)
                nc.vector.reciprocal(v5[:], v5[:])
                nc.vector.tensor_mul(t5[:], g5t[:], v5[:])
                nc.vector.tensor_sub(p5[:], pmx[:], m5[:])
                nc.vector.tensor_mul(p5[:], p5[:], t5[:])
                nc.vector.tensor_add(p5[:], p5[:], b5t[:])
                nc.vector.scalar_tensor_tensor(p5[:], p5[:], 0.2, p5[:], ALU.mult, ALU.max)
                if debug:
                    nc.sync.dma_start(dbg["d_p"][:], p5[:])

                p_d = dramp.tile([128, 16], F32, tag="pd", name="pd")
                pall_d = dramp.tile([8 * 128, 16], F32, tag="pall", name="pall")
                nc.sync.dma_start(p_d[:, :], p5[:, :])
                nc.gpsimd.collective_compute("AllGather", ALU.bypass, replica_groups=ALL8,
                                             ins=[p_d[:].opt()], outs=[pall_d[:].opt()])
                Pt = featp.tile([128, 16 * 4], F32, tag="Pt", name="Pt")
                for kb in range(16):
                    src = _ap(pall_d[:], [[16, 128], [2 * 128 * 16, 4]], offset=kb)
                    nc.sync.dma_start(Pt[:, kb * 4:(kb + 1) * 4], src)

                def mlp_layer(wt_param, nob, nkb, rhs_tile, biasp, gp, bp):
                    outt = smallp.tile([128, 4 * nob], F32, tag=f"mlp{nob}", name=f"mlp{nob}")
                    for ob in range(nob):
                        ps = mmps.tile([128, 512], F32, tag="mm", name="mm")
                        for kb in range(nkb):
                            ws = smallp.tile([128, 128], F32, tag="wmlp", name="wmlp")
                            nc.sync.dma_start(ws[:, :], wt_param[kb * 128:(kb + 1) * 128, ob * 128:(ob + 1) * 128])
                            nc.tensor.matmul(ps[:, 0:4], ws[:, :], rhs_tile[:, kb * 4:(kb + 1) * 4],
                                             start=(kb == 0), stop=(kb == nkb - 1))
                        nc.scalar.activation(outt[:, ob * 4:(ob + 1) * 4], ps[:, 0:4], AF.Identity, bias=biasp[:, ob:ob + 1])
                    mm_ = smallp.tile([128, 4], F32, tag=f"mm{nob}", name=f"mm{nob}")
                    vv_ = smallp.tile([128, 4], F32, tag=f"vv{nob}", name=f"vv{nob}")
                    sq_ = smallp.tile([128, 4 * nob], F32, tag=f"sqm{nob}", name=f"sqm{nob}")
                    t_ = smallp.tile([128, 4], F32, tag=f"t{nob}", name=f"t{nob}")
                    bb_ = smallp.tile([128, 4], F32, tag=f"bb{nob}", name=f"bb{nob}")
                    for ob in range(nob):
                        nc.vector.reduce_sum(mm_[:, ob:ob + 1], outt[:, ob * 4:(ob + 1) * 4], AX.X)
                    nc.vector.tensor_scalar_mul(mm_[:, 0:nob], mm_[:, 0:nob], 0.25)
                    nc.vector.tensor_mul(sq_[:], outt[:], outt[:])
                    for ob in range(nob):
                        nc.vector.reduce_sum(vv_[:, ob:ob + 1], sq_[:, ob * 4:(ob + 1) * 4], AX.X)
                    nc.vector.tensor_scalar_mul(vv_[:, 0:nob], vv_[:, 0:nob], 0.25)
                    nc.vector.tensor_mul(t_[:, 0:nob], mm_[:, 0:nob], mm_[:, 0:nob])
                    nc.vector.tensor_sub(vv_[:, 0:nob], vv_[:, 0:nob], t_[:, 0:nob])
                    nc.vector.tensor_scalar_add(vv_[:, 0:nob], vv_[:, 0:nob], EPS)
                    nc.scalar.activation(vv_[:, 0:nob], vv_[:, 0:nob], AF.Sqrt)
                    nc.vector.reciprocal(vv_[:, 0:nob], vv_[:, 0:nob])
                    nc.vector.tensor_mul(t_[:, 0:nob], gp[:, 0:nob], vv_[:, 0:nob])
                    nc.vector.tensor_mul(bb_[:, 0:nob], mm_[:, 0:nob], t_[:, 0:nob])
                    nc.vector.tensor_sub(bb_[:, 0:nob], bp[:, 0:nob], bb_[:, 0:nob])
                    for ob in range(nob):
                        nc.scalar.activation(outt[:, ob * 4:(ob + 1) * 4], outt[:, ob * 4:(ob + 1) * 4],
                                             AF.Relu, scale=t_[:, ob:ob + 1], bias=bb_[:, ob:ob + 1])
                    return outt

                wb6t = smallp.tile([128, 2], F32, tag="wb6t", name="wb6t")
                g6t = smallp.tile([128, 2], F32, tag="g6t", name="g6t")
                b6t = smallp.tile([128, 2], F32, tag="b6t", name="b6t")
                nc.sync.dma_start(wb6t[:], wb6[:, :])
                nc.sync.dma_start(g6t[:], g6[:, :])
                nc.sync.dma_start(b6t[:], b6[:, :])
                y6 = mlp_layer(w6t, 2, 16, Pt, wb6t, g6t, b6t)

                y6_d = dramp.tile([2 * 128, 4], F32, tag="y6d", name="y6d")
                y6a_d = dramp.tile([16 * 128, 4], F32, tag="y6a", name="y6a")
                for ob in range(2):
                    nc.sync.dma_start(y6_d[ob * 128:(ob + 1) * 128, :], y6[:, ob * 4:(ob + 1) * 4])
                nc.gpsimd.collective_compute("AllGather", ALU.bypass, replica_groups=ALL8,
                                             ins=[y6_d[:].opt()], outs=[y6a_d[:].opt()])
                r7 = featp.tile([128, 32 * 4], F32, tag="r7", name="r7")
                for kb in range(16):
                    nc.sync.dma_start(r7[:, kb * 4:(kb + 1) * 4], y6a_d[kb * 128:(kb + 1) * 128, :])
                nc.vector.tensor_copy(r7[:, 64:128], Pt[:, :])

                wb7t = smallp.tile([128, 4], F32, tag="wb7t", name="wb7t")
                g7t = smallp.tile([128, 4], F32, tag="g7t", name="g7t")
                b7t = smallp.tile([128, 4], F32, tag="b7t", name="b7t")
                nc.sync.dma_start(wb7t[:], wb7[:, :])
                nc.sync.dma_start(g7t[:], g7[:, :])
                nc.sync.dma_start(b7t[:], b7[:, :])
                y7 = mlp_layer(w7t, 4, 32, r7, wb7t, g7t, b7t)

                y7_d = dramp.tile([4 * 128, 4], F32, tag="y7d", name="y7d")
                y7a_d = dramp.tile([32 * 128, 4], F32, tag="y7a", name="y7a")
                for ob in range(4):
                    nc.sync.dma_start(y7_d[ob * 128:(ob + 1) * 128, :], y7[:, ob * 4:(ob + 1) * 4])
                nc.gpsimd.collective_compute("AllGather", ALU.bypass, replica_groups=ALL8,
                                             ins=[y7_d[:].opt()], outs=[y7a_d[:].opt()])
                r8 = featp.tile([128, 32 * 4], F32, tag="r8", name="r8")
                for kb in range(32):
                    nc.sync.dma_start(r8[:, kb * 4:(kb + 1) * 4], y7a_d[kb * 128:(kb + 1) * 128, :])

                ps8 = mmps.tile([128, 512], F32, tag="mm", name="mm")
                for kb in range(32):
                    ws = smallp.tile([128, 64], F32, tag="w8s", name="w8s")
                    nc.sync.dma_start(ws[:, :], w8t[kb * 128:(kb + 1) * 128, :])
                    nc.tensor.matmul(ps8[0:64, 0:4], ws[:, :], r8[:, kb * 4:(kb + 1) * 4],
                                     start=(kb == 0), stop=(kb == 31))
                wb8t = smallp.tile([64, 1], F32, tag="wb8t", name="wb8t")
                nc.sync.dma_start(wb8t[:], wb8[:, :])
                y8 = smallp.tile([64, 4], F32, tag="y8", name="y8")
                nc.scalar.activation(y8[:], ps8[0:64, 0:4], AF.Tanh, bias=wb8t[:, 0:1])
                nc.sync.dma_start(y_out[:, :], y8[:])

    nc.compile()
    return nc


_NC_CACHE = {}


def _wrap16(vals):
    """int16 wrapped-by-16 layout [16, n/16] replicated to [128, n/16]."""
    n = len(vals)
    w = np.zeros((16, n // 16), np.int16)
    for i, v in enumerate(vals):
        w[i % 16, i // 16] = v
    return np.tile(w, (8, 1))


def kernel(**inputs):
    x = np.asarray(inputs["x"], dtype=np.float32)
    w = {k: np.asarray(v, dtype=np.float32) for k, v in inputs.items()}

    def pl(a, cols):
        a = np.asarray(a, np.float32).reshape(-1)
        out = np.zeros((128, cols), np.float32)
        out[np.arange(a.size) % 128, np.arange(a.size) // 128] = a
        return out

    base = {}
    for li, nm in enumerate(["1", "2", "3", "4"]):
        C, O = CS[li], OS[li]
        nt = max(1, O // 128)
        wl = w[f"w{nm}"]
        base[f"wdt{li}"] = np.ascontiguousarray(wl[:, :C].T)
        base[f"wvt{li}"] = np.ascontiguousarray((wl[:, C:] - wl[:, :C]).T)
        base[f"g{li}"] = pl(w[f"g{nm}"], nt)
        base[f"b{li}"] = pl(w[f"b{nm}"], nt)
    w5 = w["w5"]
    offs = [0, 3, 67, 195, 451, 963]
    for j in range(5):
        base[f"w5t{j}"] = np.ascontiguousarray(w5[:, offs[j]:offs[j + 1]].T)
    base["g5"] = pl(w["g5"], 16)
    base["b5"] = pl(w["b5"], 16)
    base["ident"] = np.eye(128, dtype=np.float32)

    in_maps = []
    for c in range(8):
        b, h = c // 2, c % 2
        m = dict(base)
        xs = x[b]
        if h == 1:
            xs = np.concatenate([xs[:, NH:], xs[:, :NH]], axis=1)
        m["x"] = np.ascontiguousarray(xs)
        for li in range(3):
            nt = max(1, OS[li] // 128)
            sel = np.zeros((128, 2 * nt * 8), np.int16)
            for hh in range(2):
                for ot in range(nt):
                    rows = ((h ^ hh) * nt + ot) * 128 + np.arange(128)
                    sel[:, (hh * nt + ot) * 8:(hh * nt + ot + 1) * 8] = _wrap16(list(rows))
            m[f"agsel{li}"] = sel
        m["w6t"] = np.ascontiguousarray(w["w6"][c * 256:(c + 1) * 256, :].T)
        m["wb6"] = pl(w["wb6"][c * 256:(c + 1) * 256], 2)
        m["g6"] = pl(w["g6"][c * 256:(c + 1) * 256], 2)
        m["b6"] = pl(w["b6"][c * 256:(c + 1) * 256], 2)
        m["w7t"] = np.ascontiguousarray(w["w7"][c * 512:(c + 1) * 512, :].T)
        m["wb7"] = pl(w["wb7"][c * 512:(c + 1) * 512], 4)
        m["g7"] = pl(w["g7"][c * 512:(c + 1) * 512], 4)
        m["b7"] = pl(w["b7"][c * 512:(c + 1) * 512], 4)
        w8pad = np.zeros((64, 4096), np.float32)
        w8pad[:50] = w["w8"][c * 50:(c + 1) * 50, :]
        m["w8t"] = np.ascontiguousarray(w8pad.T)
        wb8pad = np.zeros((64, 1), np.float32)
        wb8pad[:50, 0] = w["wb8"][c * 50:(c + 1) * 50]
        m["wb8"] = wb8pad
        in_maps.append(m)

    key = ("k", DEBUG)
    if key not in _NC_CACHE:
        _NC_CACHE[key] = build(debug=DEBUG)
    nc = _NC_CACHE[key]
    res = run_bass_kernel_spmd(nc, in_maps, list(range(8)))
    kernel.last_results = res
    out = np.concatenate([res.results[c]["y_out"][:50, :] for c in range(8)], axis=0)
    return np.ascontiguousarray(out.T.astype(np.float32))


if __name__ == "__main__":
    _d = np.load("/root/problem/ref_data.npz")
    inputs = {k: _d[k] for k in _d.files if k != "expected"}
    out = kernel(**inputs)
    exp = _d["expected"]
    rel = np.abs(out - exp).max() / np.abs(exp).max()
    print("Relative error:", rel)



# revision 2
# speedup vs baseline: 1372.1687x; 1372.1687x over previous
"""DGCNN control-points kernel on 8 trn2 NeuronCores.

Sharding: core c -> sample b=c//2, half h=c%2. Each core's x is host-rotated so
its own 1024 points sit in columns 0:1024 (pair AllReduce + reload restores the
other half). Edge conv y = u[.,idx] + v with u=w[:, :C]@x,
v=(w[:,C:]-w[:, :C])@x; BN+lrelu commute with max over k.

kNN via packed top-k: p = round(Relu(md*S + 4096)) (scalar engine, int16),
packed = p*2048 + col_index (DVE STT, fp32-exact ints < 2^24), then per-256-col
group max8 (top-8 each) and a 3-round max8/match_replace playoff over the 64
candidates; index = packed mod 2048. BN stats: per-block gathered sums s1 =
sum_k u[idx], cross = s1*v (STT accum), s2 = sum u[idx]^2 via scalar Square
accum_out; sum_y = s1 + K*sum v, sum_y2 = s2 + 2*cross + K*sum v^2.
Gather: gpsimd ap_gather of u rows with wrapped-16 idx layout.
"""
import os
import sys

sys.path.insert(0, "/opt/trn_rl_repo")
import numpy as np
import concourse.bass as bass
import concourse.bacc as bacc
import concourse.tile as tile
from concourse import mybir
from concourse.bass_utils import run_bass_kernel_spmd

AF = mybir.ActivationFunctionType
ALU = mybir.AluOpType
AX = mybir.AxisListType
F32 = mybir.dt.float32
U16 = mybir.dt.uint16
I16 = mybir.dt.int16
I32 = mybir.dt.int32

B, N, K, NCP = 4, 2048, 20, 400
NH = N // 2
NB = NH // 128
EPS = 1e-5
CS = [3, 64, 128, 256]
OS = [64, 128, 256, 512]
CNT_TOT = float(B * N * K)
DEBUG = bool(os.environ.get("DGCNN_DEBUG"))
SKIPCOLL = int(os.environ.get("DGCNN_SKIPCOLL", "0"))

PAIRS = [[0, 1], [2, 3], [4, 5], [6, 7]]
ALL8 = [list(range(8))]


def _ap(t, dims, offset=0):
    a = t[:] if not isinstance(t, bass.AP) else t
    return bass.AP(tensor=a.tensor, offset=a.offset + offset, ap=list(dims))


def build(debug=False):
    nc = bacc.Bacc("TRN2", target_bir_lowering=False, debug=False, num_devices=8)
    dp = lambda n_, s, d=F32: nc.declare_dram_parameter(n_, s, d, isOutput=False)

    x_in = dp("x", [3, N])
    iota_in = dp("iota", [128, N], I16)
    wdt, wvt, gam, bet = [], [], [], []
    for li in range(4):
        C, O = CS[li], OS[li]
        nt = max(1, O // 128)
        wdt.append(dp(f"wdt{li}", [C, O]))
        wvt.append(dp(f"wvt{li}", [C, O]))
        gam.append(dp(f"g{li}", [128, nt]))
        bet.append(dp(f"b{li}", [128, nt]))
    w5t = [dp(f"w5t{j}", [c, 2048]) for j, c in enumerate([3, 64, 128, 256, 512])]
    g5 = dp("g5", [128, 16])
    b5 = dp("b5", [128, 16])
    w6t = dp("w6t", [2048, 256])
    wb6 = dp("wb6", [128, 2])
    g6 = dp("g6", [128, 2])
    b6 = dp("b6", [128, 2])
    w7t = dp("w7t", [4096, 512])
    wb7 = dp("wb7", [128, 4])
    g7 = dp("g7", [128, 4])
    b7 = dp("b7", [128, 4])
    w8t = dp("w8t", [4096, 64])
    wb8 = dp("wb8", [64, 1])

    y_out = nc.declare_dram_parameter("y_out", [64, B], F32, isOutput=True)
    dbg = {}
    if debug:
        for nm, shp in [("d_x2", [1, N]), ("d_md", [128, N]), ("d_idx", [128, 24]),
                        ("d_x1", [128, N]), ("d_x4", [128, NH]), ("d_p", [128, 16]),
                        ("d_stats", [128, 8])]:
            dbg[nm] = nc.declare_dram_parameter(nm, shp, F32, isOutput=True)

    with tile.TileContext(nc) as tc:
        with (
            tc.tile_pool(name="feat", bufs=1) as featp,
            tc.tile_pool(name="work", bufs=2) as workp,
            tc.tile_pool(name="small", bufs=2) as smallp,
            tc.tile_pool(name="const", bufs=1) as constp,
            tc.tile_pool(name="mdps", bufs=2, space="PSUM") as mdps,
            tc.tile_pool(name="mm", bufs=2, space="PSUM") as mmps,
            tc.tile_pool(name="dram", bufs=1, space="DRAM") as dramp,
        ):
            ones_col = constp.tile([128, 1], F32)
            nc.vector.memset(ones_col[:], 1.0)
            ones_row = constp.tile([1, 128], F32)
            nc.vector.memset(ones_row[:], 1.0)
            x2sb = constp.tile([1, N], F32)
            lhs2 = constp.tile([2, N], F32)
            rhs2 = constp.tile([2, N], F32)
            nc.vector.memset(lhs2[0:1, :], 1.0)
            nc.vector.memset(rhs2[0:2, :], -0.5)
            iota_f = constp.tile([128, N], I16)
            nc.sync.dma_start(iota_f[:], iota_in[:, :])

            ut_d = dramp.tile([N, 512], F32, tag="utd", name="utd")
            idx_d = dramp.tile([NB * 128, 20], I16, tag="idxd", name="idxd")
            agi_d = dramp.tile([256, 1024], F32, tag="agin", name="agin")
            ago_d = dramp.tile([512, 1024], F32, tag="agout", name="agout")
            st_d = dramp.tile([128, 8], F32, tag="stin", name="stin")
            st2_d = dramp.tile([128, 8], F32, tag="stout", name="stout")

            x_t = {0: [featp.tile([128, N], F32, tag="x0", name="x0")]}
            nc.sync.dma_start(x_t[0][0][0:3, :], x_in[:, :])

            for li in range(4):
                C, O = CS[li], OS[li]
                nct = max(1, C // 128)
                not_ = max(1, O // 128)
                xt = x_t[li]
                crows = [min(128, C - 128 * t) for t in range(nct)]

                # ---- x2 row + quantization scale ----
                for j in range(4):
                    x2ps = mmps.tile([128, 512], F32, tag="mm", name="mm")
                    for t in range(nct):
                        r = crows[t]
                        sqc = workp.tile([128, 512], F32, tag="sqc", name="sqc")
                        nc.scalar.activation(sqc[0:r, :], xt[t][0:r, j * 512:(j + 1) * 512], AF.Square)
                        nc.tensor.matmul(x2ps[0:1, :], ones_col[0:r, :],
                                         sqc[0:r, :],
                                         start=(t == 0), stop=(t == nct - 1))
                    nc.scalar.activation(x2sb[0:1, j * 512:(j + 1) * 512], x2ps[0:1, :], AF.Copy)
                nc.scalar.activation(rhs2[0:1, :], x2sb[0:1, :], AF.Copy, scale=-0.5)
                nc.sync.dma_start(lhs2[1:2, :], x2sb[0:1, :])
                if debug and li == 0:
                    nc.sync.dma_start(dbg["d_x2"][:], x2sb[0:1, :])
                # x2max -> S = 2048/x2max broadcast to [128,1]
                x2m = smallp.tile([1, 1], F32, tag="x2m", name="x2m")
                nc.vector.reduce_max(x2m[0:1, 0:1], x2sb[0:1, :], AX.X)
                x2mp = mmps.tile([128, 512], F32, tag="mm", name="mm")
                nc.tensor.matmul(x2mp[:, 0:1], ones_row[0:1, :], x2m[0:1, 0:1],
                                 start=True, stop=True)
                s_col = smallp.tile([128, 2], F32, tag="scol", name="scol")
                nc.scalar.activation(s_col[:, 1:2], x2mp[:, 0:1], AF.Copy)
                nc.vector.reciprocal(s_col[:, 1:2], s_col[:, 1:2])
                nc.vector.tensor_scalar_mul(s_col[:, 0:1], s_col[:, 1:2], 2048.0)

                # ---- v [O-part, NH] (+ Sv accumulated via copy) ----
                v_t = [featp.tile([128, NH], F32, tag=f"v{t}", name=f"v{t}") for t in range(not_)]
                svc = smallp.tile([128, 4 * 2], F32, tag="svc", name="svc", bufs=1)
                for ot in range(not_):
                    oc = min(128, O - 128 * ot)
                    wvs = [smallp.tile([128, 128], F32, tag=f"wsl{t}", name=f"wsl{t}", bufs=1) for t in range(nct)]
                    for t in range(nct):
                        r = crows[t]
                        nc.sync.dma_start(wvs[t][0:r, 0:oc], wvt[li][128 * t:128 * t + r, 128 * ot:128 * ot + oc])
                    for j in range(NH // 512):
                        vps = mmps.tile([128, 512], F32, tag="mm", name="mm")
                        for t in range(nct):
                            r = crows[t]
                            nc.tensor.matmul(vps[0:oc, :], wvs[t][0:r, 0:oc],
                                             xt[t][0:r, j * 512:(j + 1) * 512],
                                             start=(t == 0), stop=(t == nct - 1))
                        nc.scalar.activation(v_t[ot][0:oc, j * 512:(j + 1) * 512], vps[0:oc, :], AF.Copy,
                                             accum_out=svc[0:oc, 2 * ot + j:2 * ot + j + 1])

                # ---- u [O-part, N] in SBUF (gather source) ----
                wds_g = []
                for ot in range(not_):
                    oc = min(128, O - 128 * ot)
                    row = [smallp.tile([128, 128], F32, tag=f"wsl2{t}", name=f"wsl2{t}", bufs=1) for t in range(nct)]
                    for t in range(nct):
                        r = crows[t]
                        nc.sync.dma_start(row[t][0:r, 0:oc], wdt[li][128 * t:128 * t + r, 128 * ot:128 * ot + oc])
                    wds_g.append(row)
                u_t = [featp.tile([128, N], F32, tag=f"u{t}", name=f"u{t}") for t in range(not_)]
                for ot in range(not_):
                    oc = min(128, O - 128 * ot)
                    for j in range(N // 512):
                        ups = mmps.tile([128, 512], F32, tag="mm", name="mm")
                        for t in range(nct):
                            r = crows[t]
                            nc.tensor.matmul(ups[0:oc, :], wds_g[ot][t][0:r, 0:oc],
                                             xt[t][0:r, j * 512:(j + 1) * 512],
                                             start=(t == 0), stop=(t == nct - 1))
                        nc.scalar.activation(u_t[ot][0:oc, j * 512:(j + 1) * 512], ups[0:oc, :], AF.Copy)

                # ---- per-block: packed distances, grouped top-k, idx spill ----
                idxw = featp.tile([128, NB * 160], I16, tag="idxw", name="idxw")
                for nb in range(NB):
                    pk = workp.tile([128, N], I16, tag="pk", name="pk")
                    for j in range(4):
                        js = slice(j * 512, (j + 1) * 512)
                        mdj = mdps.tile([128, 512], F32, tag="md", name="md")
                        for t in range(nct):
                            r = crows[t]
                            nc.tensor.matmul(mdj[:, :], xt[t][0:r, nb * 128:(nb + 1) * 128],
                                             xt[t][0:r, js], start=(t == 0), stop=False)
                        nc.tensor.matmul(mdj[:, :], lhs2[0:2, nb * 128:(nb + 1) * 128],
                                         rhs2[0:2, js], start=False, stop=True)
                        nc.scalar.activation(pk[:, js], mdj[:, :], AF.Relu,
                                             scale=s_col[:, 0:1], bias=4096.0)
                    pck = workp.tile([128, N], F32, tag="pck", name="pck")
                    nc.vector.scalar_tensor_tensor(pck[:, :], pk[:, :], 2048.0, iota_f[:, :],
                                                   ALU.mult, ALU.add)
                    if debug and li == 0 and nb == 0:
                        nc.sync.dma_start(dbg["d_md"][:], pck[:])
                    m8all = smallp.tile([128, 64], F32, tag="m8all", name="m8all")
                    for g in range(8):
                        nc.vector.max(m8all[:, 8 * g:8 * (g + 1)], pck[:, 256 * g:256 * (g + 1)])
                    c24 = smallp.tile([128, 24], F32, tag="c24", name="c24")
                    for rnd in range(3):
                        nc.vector.max(c24[:, 8 * rnd:8 * (rnd + 1)], m8all[:, :])
                        if rnd < 2:
                            nc.vector.match_replace(m8all[:, :], c24[:, 8 * rnd:8 * (rnd + 1)],
                                                    m8all[:, :], -3.0e38)
                    idxf = smallp.tile([128, 20], F32, tag="idxf", name="idxf")
                    nc.vector.tensor_scalar(idxf[:, :], c24[:, 0:20], 2048.0, None, ALU.mod)
                    if debug and li == 0 and nb == 0:
                        nc.sync.dma_start(dbg["d_idx"][:, 0:20], idxf[:])
                    idx20i = smallp.tile([128, 20], I16, tag="idx20i", name="idx20i")
                    nc.vector.tensor_copy(idx20i[:], idxf[:])
                    nc.sync.dma_start(idx_d[nb * 128:(nb + 1) * 128, :], idx20i[:])
                    # wrapped reload (replicated to all 8 gpsimd core groups)
                    src = _ap(idx_d[:], [[0, 8], [1, 16], [16, 160]], offset=nb * 2560)
                    nc.sync.dma_start(idxw[:, nb * 160:(nb + 1) * 160], src)

                # ---- gathers + reduces ([O,(n,k)] layout) ----
                if li < 3:
                    xn = [featp.tile([128, N], F32, tag=f"x{li + 1}_{t}", name=f"x{li + 1}_{t}") for t in range(not_)]
                else:
                    xn = [featp.tile([128, NH], F32, tag=f"x4_{t}", name=f"x4_{t}") for t in range(not_)]
                s1c = smallp.tile([128, 4 * NB], F32, tag="s1c", name="s1c", bufs=1)
                crc = smallp.tile([128, 4 * NB], F32, tag="crc", name="crc", bufs=1)
                s2c = smallp.tile([128, 4 * NB], F32, tag="s2c", name="s2c", bufs=1)
                for nb in range(NB):
                    for ot in range(not_):
                        oc = min(128, O - 128 * ot)
                        gat = workp.tile([128, 2560], F32, tag="gat", name="gat")
                        nc.gpsimd.ap_gather(gat[0:oc, :], u_t[ot][0:oc, :],
                                            idxw[0:oc, nb * 160:(nb + 1) * 160],
                                            channels=oc if oc >= 16 else 16,
                                            num_elems=N, d=1, num_idxs=2560)
                        gap = bass.AP(tensor=gat[:].tensor, offset=gat[:].offset,
                                      ap=[[list(gat[:].ap[0])[0], oc]] + [[20, 128], [1, 20]])
                        nc.vector.tensor_reduce(xn[ot][0:oc, nb * 128:(nb + 1) * 128], gap, AX.X, ALU.max)
                        sblk = workp.tile([128, 128], F32, tag="sblk", name="sblk")
                        nc.vector.tensor_reduce(sblk[0:oc, :], gap, AX.X, ALU.add)
                        nc.vector.reduce_sum(s1c[0:oc, ot * NB + nb:ot * NB + nb + 1], sblk[0:oc, :], AX.X)
                        prodb = workp.tile([128, 128], F32, tag="prodb", name="prodb")
                        nc.vector.scalar_tensor_tensor(prodb[0:oc, :], sblk[0:oc, :], 1.0,
                                                       v_t[ot][0:oc, nb * 128:(nb + 1) * 128],
                                                       ALU.mult, ALU.mult,
                                                       accum_out=crc[0:oc, ot * NB + nb:ot * NB + nb + 1])
                        sqd = workp.tile([128, 2560], F32, tag="sqd", name="sqd", bufs=1)
                        nc.scalar.activation(sqd[0:oc, :], gat[0:oc, :], AF.Square,
                                             accum_out=s2c[0:oc, ot * NB + nb:ot * NB + nb + 1])

                # ---- stats assembly ----
                stat = smallp.tile([128, 8], F32, tag="stat", name="stat")
                scr = smallp.tile([128, 12], F32, tag="scr", name="scr")
                for ot in range(not_):
                    oc = min(128, O - 128 * ot)
                    nc.vector.tensor_add(xn[ot][0:oc, 0:NH], xn[ot][0:oc, 0:NH], v_t[ot][0:oc, :])
                    nc.vector.reduce_sum(scr[0:oc, 0:1], s1c[0:oc, ot * NB:(ot + 1) * NB], AX.X)
                    nc.vector.reduce_sum(scr[0:oc, 1:2], crc[0:oc, ot * NB:(ot + 1) * NB], AX.X)
                    nc.vector.reduce_sum(scr[0:oc, 2:3], s2c[0:oc, ot * NB:(ot + 1) * NB], AX.X)
                    nc.vector.tensor_add(scr[0:oc, 3:4], svc[0:oc, 2 * ot:2 * ot + 1], svc[0:oc, 2 * ot + 1:2 * ot + 2])
                    # Sv2
                    tmp = workp.tile([128, NH], F32, tag="stmp", name="stmp", bufs=1)
                    nc.scalar.activation(tmp[0:oc, :], v_t[ot][0:oc, :], AF.Square,
                                         accum_out=scr[0:oc, 4:5])
                    # sum_y = s1 + K*Sv
                    nc.vector.scalar_tensor_tensor(stat[0:oc, 2 * ot:2 * ot + 1], scr[0:oc, 3:4],
                                                   float(K), scr[0:oc, 0:1], ALU.mult, ALU.add)
                    # sum_y2 = s2 + 2*cross + K*Sv2
                    nc.vector.scalar_tensor_tensor(scr[0:oc, 5:6], scr[0:oc, 1:2], 2.0,
                                                   scr[0:oc, 2:3], ALU.mult, ALU.add)
                    nc.vector.scalar_tensor_tensor(stat[0:oc, 2 * ot + 1:2 * ot + 2], scr[0:oc, 4:5],
                                                   float(K), scr[0:oc, 5:6], ALU.mult, ALU.add)

                nc.sync.dma_start(st_d[:, :], stat[:, :])
                if SKIPCOLL:
                    nc.gpsimd.dma_start(st2_d[:, :], st_d[:, :])
                else:
                    nc.gpsimd.collective_compute("AllReduce", ALU.add, replica_groups=ALL8,
                                                 ins=[st_d[:].opt()], outs=[st2_d[:].opt()])
                statg = smallp.tile([128, 8], F32, tag="statg", name="statg")
                nc.sync.dma_start(statg[:], st2_d[:, :])
                if debug and li == 0:
                    nc.sync.dma_start(dbg["d_stats"][:], statg[:])

                # bn scale/bias per O-tile
                gpt = smallp.tile([128, 4], F32, tag="gpt", name="gpt")
                bpt = smallp.tile([128, 4], F32, tag="bpt", name="bpt")
                nc.sync.dma_start(gpt[:, 0:not_], gam[li][:, :])
                nc.sync.dma_start(bpt[:, 0:not_], bet[li][:, :])
                mean = smallp.tile([128, 4], F32, tag="mean", name="mean")
                var = smallp.tile([128, 4], F32, tag="var", name="var")
                scl = smallp.tile([128, 4], F32, tag="scl", name="scl")
                bia = smallp.tile([128, 4], F32, tag="bia", name="bia")
                for ot in range(not_):
                    nc.vector.tensor_scalar_mul(mean[:, ot:ot + 1], statg[:, 2 * ot:2 * ot + 1], 1.0 / CNT_TOT)
                    nc.vector.tensor_scalar_mul(var[:, ot:ot + 1], statg[:, 2 * ot + 1:2 * ot + 2], 1.0 / CNT_TOT)
                nc.vector.tensor_mul(scl[:, 0:not_], mean[:, 0:not_], mean[:, 0:not_])
                nc.vector.tensor_sub(var[:, 0:not_], var[:, 0:not_], scl[:, 0:not_])
                nc.vector.tensor_scalar_add(var[:, 0:not_], var[:, 0:not_], EPS)
                nc.scalar.activation(var[:, 0:not_], var[:, 0:not_], AF.Sqrt)
                nc.vector.reciprocal(var[:, 0:not_], var[:, 0:not_])
                nc.vector.tensor_mul(scl[:, 0:not_], gpt[:, 0:not_], var[:, 0:not_])
                nc.vector.tensor_mul(bia[:, 0:not_], mean[:, 0:not_], scl[:, 0:not_])
                nc.vector.tensor_sub(bia[:, 0:not_], bpt[:, 0:not_], bia[:, 0:not_])

                if li < 3:
                    for ot in range(not_):
                        nc.sync.dma_start(agi_d[ot * 128:(ot + 1) * 128, :], xn[ot][:, 0:NH])
                    if SKIPCOLL:
                        nc.gpsimd.dma_start(ago_d[0:not_ * 128, :], agi_d[0:not_ * 128, :])
                    else:
                        nc.gpsimd.collective_compute(
                            "AllReduce", ALU.add, replica_groups=PAIRS,
                            ins=[agi_d[0:not_ * 128, :].opt()], outs=[ago_d[0:not_ * 128, :].opt()])
                    for ot in range(not_):
                        nc.sync.dma_start(xn[ot][:, NH:N], ago_d[ot * 128:(ot + 1) * 128, :])
                        nc.vector.tensor_sub(xn[ot][:, NH:N], xn[ot][:, NH:N], xn[ot][:, 0:NH])
                        nc.scalar.activation(xn[ot][:, :], xn[ot][:, :], AF.Identity,
                                             scale=scl[:, ot:ot + 1], bias=bia[:, ot:ot + 1])
                        nc.vector.scalar_tensor_tensor(xn[ot][:, :], xn[ot][:, :], 0.2,
                                                       xn[ot][:, :], ALU.mult, ALU.max)
                    x_t[li + 1] = xn
                else:
                    for ot in range(not_):
                        nc.scalar.activation(xn[ot][:, :], xn[ot][:, :], AF.Identity,
                                             scale=scl[:, ot:ot + 1], bias=bia[:, ot:ot + 1])
                        nc.vector.scalar_tensor_tensor(xn[ot][:, :], xn[ot][:, :], 0.2,
                                                       xn[ot][:, :], ALU.mult, ALU.max)
                    x_t[4] = xn
                if debug and li == 0:
                    nc.sync.dma_start(dbg["d_x1"][:], x_t[1][0][:, :])
                if debug and li == 3:
                    nc.sync.dma_start(dbg["d_x4"][:], x_t[4][0][:, :])

            # ---------------- L5 + pool ----------------
            hsrc = [(x_t[0][0], 3, w5t[0], 0), (x_t[1][0], 64, w5t[1], 0),
                    (x_t[2][0], 128, w5t[2], 0), (x_t[3][0], 128, w5t[3], 0),
                    (x_t[3][1], 128, w5t[3], 128)]
            for t in range(4):
                hsrc.append((x_t[4][t], 128, w5t[4], 128 * t))
            s1t = smallp.tile([128, 16], F32, tag="s1t", name="s1t")
            s2t = smallp.tile([128, 16], F32, tag="s2t", name="s2t")
            pmx = smallp.tile([128, 16], F32, tag="pmx", name="pmx")
            acc1 = smallp.tile([128, 2], F32, tag="acc1", name="acc1")
            for ob in range(16):
                wss = []
                for si, (xt_, r, wp, roff) in enumerate(hsrc):
                    ws = smallp.tile([128, 128], F32, tag=f"w5s{si}", name=f"w5s{si}")
                    nc.sync.dma_start(ws[0:r, :], wp[roff:roff + r, ob * 128:(ob + 1) * 128])
                    wss.append(ws)
                h5s = workp.tile([128, NH], F32, tag="h5s", name="h5s", bufs=1)
                for jj in range(2):
                    h5ps = mmps.tile([128, 512], F32, tag="mm", name="mm")
                    js = slice(jj * 512, jj * 512 + 512)
                    for si, (xt_, r, wp, roff) in enumerate(hsrc):
                        nc.tensor.matmul(h5ps[:, :], wss[si][0:r, :], xt_[0:r, js],
                                         start=(si == 0), stop=(si == len(hsrc) - 1))
                    nc.scalar.activation(h5s[:, js], h5ps[:, :], AF.Copy, accum_out=acc1[:, jj:jj + 1])
                nc.vector.tensor_add(s1t[:, ob:ob + 1], acc1[:, 0:1], acc1[:, 1:2])
                sq5 = workp.tile([128, NH], F32, tag="sq5", name="sq5", bufs=1)
                nc.scalar.activation(sq5[:, :], h5s[:, :], AF.Square, accum_out=s2t[:, ob:ob + 1])
                nc.vector.reduce_max(pmx[:, ob:ob + 1], h5s[:, :], AX.X)

            s12 = smallp.tile([128, 32], F32, tag="s12", name="s12")
            nc.vector.tensor_copy(s12[:, 0:16], s1t[:, :])
            nc.vector.tensor_copy(s12[:, 16:32], s2t[:, :])
            st32_d = dramp.tile([128, 32], F32, tag="st32", name="st32")
            st32o_d = dramp.tile([128, 32], F32, tag="st32o", name="st32o")
            nc.sync.dma_start(st32_d[:, :], s12[:, :])
            nc.gpsimd.collective_compute("AllReduce", ALU.add, replica_groups=ALL8,
                                         ins=[st32_d[:].opt()], outs=[st32o_d[:].opt()])
            nc.sync.dma_start(s12[:, :], st32o_d[:, :])
            pm_d = dramp.tile([128, 16], F32, tag="pmd", name="pmd")
            pm2_d = dramp.tile([128, 16], F32, tag="pmd2", name="pmd2")
            nc.sync.dma_start(pm_d[:, :], pmx[:, :])
            nc.gpsimd.collective_compute("AllReduce", ALU.max, replica_groups=PAIRS,
                                         ins=[pm_d[:].opt()], outs=[pm2_d[:].opt()])
            nc.sync.dma_start(pmx[:, :], pm2_d[:, :])

            g5t = smallp.tile([128, 16], F32, tag="g5t", name="g5t")
            b5t = smallp.tile([128, 16], F32, tag="b5t", name="b5t")
            nc.sync.dma_start(g5t[:], g5[:, :])
            nc.sync.dma_start(b5t[:], b5[:, :])
            m5 = smallp.tile([128, 16], F32, tag="m5", name="m5")
            v5 = smallp.tile([128, 16], F32, tag="v5", name="v5")
            t5 = smallp.tile([128, 16], F32, tag="t5", name="t5")
            p5 = smallp.tile([128, 16], F32, tag="p5", name="p5")
            nc.vector.tensor_scalar_mul(m5[:], s12[:, 0:16], 1.0 / (B * N))
            nc.vector.tensor_scalar_mul(v5[:], s12[:, 16:32], 1.0 / (B * N))
            nc.vector.tensor_mul(t5[:], m5[:], m5[:])
            nc.vector.tensor_sub(v5[:], v5[:], t5[:])
            nc.vector.tensor_scalar_add(v5[:], v5[:], EPS)
            nc.scalar.activation(v5[:], v5[:], AF.Sqrt)
            nc.vector.reciprocal(v5[:], v5[:])
            nc.vector.tensor_mul(t5[:], g5t[:], v5[:])
            nc.vector.tensor_sub(p5[:], pmx[:], m5[:])
            nc.vector.tensor_mul(p5[:], p5[:], t5[:])
            nc.vector.tensor_add(p5[:], p5[:], b5t[:])
            nc.vector.scalar_tensor_tensor(p5[:], p5[:], 0.2, p5[:], ALU.mult, ALU.max)
            if debug:
                nc.sync.dma_start(dbg["d_p"][:], p5[:])

            p_d = dramp.tile([128, 16], F32, tag="pd", name="pd")
            pall_d = dramp.tile([8 * 128, 16], F32, tag="pall", name="pall")
            nc.sync.dma_start(p_d[:, :], p5[:, :])
            nc.gpsimd.collective_compute("AllGather", ALU.bypass, replica_groups=ALL8,
                                         ins=[p_d[:].opt()], outs=[pall_d[:].opt()])
            Pt = featp.tile([128, 16 * 4], F32, tag="Pt", name="Pt")
            for kb in range(16):
                src = _ap(pall_d[:], [[16, 128], [2 * 128 * 16, 4]], offset=kb)
                nc.sync.dma_start(Pt[:, kb * 4:(kb + 1) * 4], src)

            def mlp_layer(wt_param, nob, nkb, rhs_tile, biasp, gp, bp):
                outt = smallp.tile([128, 4 * nob], F32, tag=f"mlp{nob}", name=f"mlp{nob}")
                for ob in range(nob):
                    ps = mmps.tile([128, 512], F32, tag="mm", name="mm")
                    for kb in range(nkb):
                        ws = smallp.tile([128, 128], F32, tag="wmlp", name="wmlp")
                        nc.sync.dma_start(ws[:, :], wt_param[kb * 128:(kb + 1) * 128, ob * 128:(ob + 1) * 128])
                        nc.tensor.matmul(ps[:, 0:4], ws[:, :], rhs_tile[:, kb * 4:(kb + 1) * 4],
                                         start=(kb == 0), stop=(kb == nkb - 1))
                    nc.scalar.activation(outt[:, ob * 4:(ob + 1) * 4], ps[:, 0:4], AF.Identity, bias=biasp[:, ob:ob + 1])
                mm_ = smallp.tile([128, 4], F32, tag=f"mm{nob}", name=f"mm{nob}")
                vv_ = smallp.tile([128, 4], F32, tag=f"vv{nob}", name=f"vv{nob}")
                sq_ = smallp.tile([128, 4 * nob], F32, tag=f"sqm{nob}", name=f"sqm{nob}")
                t_ = smallp.tile([128, 4], F32, tag=f"t{nob}", name=f"t{nob}")
                bb_ = smallp.tile([128, 4], F32, tag=f"bb{nob}", name=f"bb{nob}")
                for ob in range(nob):
                    nc.vector.reduce_sum(mm_[:, ob:ob + 1], outt[:, ob * 4:(ob + 1) * 4], AX.X)
                nc.vector.tensor_scalar_mul(mm_[:, 0:nob], mm_[:, 0:nob], 0.25)
                nc.vector.tensor_mul(sq_[:], outt[:], outt[:])
                for ob in range(nob):
                    nc.vector.reduce_sum(vv_[:, ob:ob + 1], sq_[:, ob * 4:(ob + 1) * 4], AX.X)
                nc.vector.tensor_scalar_mul(vv_[:, 0:nob], vv_[:, 0:nob], 0.25)
                nc.vector.tensor_mul(t_[:, 0:nob], mm_[:, 0:nob], mm_[:, 0:nob])
                nc.vector.tensor_sub(vv_[:, 0:nob], vv_[:, 0:nob], t_[:, 0:nob])
                nc.vector.tensor_scalar_add(vv_[:, 0:nob], vv_[:, 0:nob], EPS)
                nc.scalar.activation(vv_[:, 0:nob], vv_[:, 0:nob], AF.Sqrt)
                nc.vector.reciprocal(vv_[:, 0:nob], vv_[:, 0:nob])
                nc.vector.tensor_mul(t_[:, 0:nob], gp[:, 0:nob], vv_[:, 0:nob])
                nc.vector.tensor_mul(bb_[:, 0:nob], mm_[:, 0:nob], t_[:, 0:nob])
                nc.vector.tensor_sub(bb_[:, 0:nob], bp[:, 0:nob], bb_[:, 0:nob])
                for ob in range(nob):
                    nc.scalar.activation(outt[:, ob * 4:(ob + 1) * 4], outt[:, ob * 4:(ob + 1) * 4],
                                         AF.Relu, scale=t_[:, ob:ob + 1], bias=bb_[:, ob:ob + 1])
                return outt

            wb6t = smallp.tile([128, 2], F32, tag="wb6t", name="wb6t")
            g6t = smallp.tile([128, 2], F32, tag="g6t", name="g6t")
            b6t = smallp.tile([128, 2], F32, tag="b6t", name="b6t")
            nc.sync.dma_start(wb6t[:], wb6[:, :])
            nc.sync.dma_start(g6t[:], g6[:, :])
            nc.sync.dma_start(b6t[:], b6[:, :])
            y6 = mlp_layer(w6t, 2, 16, Pt, wb6t, g6t, b6t)

            y6_d = dramp.tile([2 * 128, 4], F32, tag="y6d", name="y6d")
            y6a_d = dramp.tile([16 * 128, 4], F32, tag="y6a", name="y6a")
            for ob in range(2):
                nc.sync.dma_start(y6_d[ob * 128:(ob + 1) * 128, :], y6[:, ob * 4:(ob + 1) * 4])
            nc.gpsimd.collective_compute("AllGather", ALU.bypass, replica_groups=ALL8,
                                         ins=[y6_d[:].opt()], outs=[y6a_d[:].opt()])
            r7 = featp.tile([128, 32 * 4], F32, tag="r7", name="r7")
            for kb in range(16):
                nc.sync.dma_start(r7[:, kb * 4:(kb + 1) * 4], y6a_d[kb * 128:(kb + 1) * 128, :])
            nc.vector.tensor_copy(r7[:, 64:128], Pt[:, :])

            wb7t = smallp.tile([128, 4], F32, tag="wb7t", name="wb7t")
            g7t = smallp.tile([128, 4], F32, tag="g7t", name="g7t")
            b7t = smallp.tile([128, 4], F32, tag="b7t", name="b7t")
            nc.sync.dma_start(wb7t[:], wb7[:, :])
            nc.sync.dma_start(g7t[:], g7[:, :])
            nc.sync.dma_start(b7t[:], b7[:, :])
            y7 = mlp_layer(w7t, 4, 32, r7, wb7t, g7t, b7t)

            y7_d = dramp.tile([4 * 128, 4], F32, tag="y7d", name="y7d")
            y7a_d = dramp.tile([32 * 128, 4], F32, tag="y7a", name="y7a")
            for ob in range(4):
                nc.sync.dma_start(y7_d[ob * 128:(ob + 1) * 128, :], y7[:, ob * 4:(ob + 1) * 4])
            nc.gpsimd.collective_compute("AllGather", ALU.bypass, replica_groups=ALL8,
                                         ins=[y7_d[:].opt()], outs=[y7a_d[:].opt()])
            r8 = featp.tile([128, 32 * 4], F32, tag="r8", name="r8")
            for kb in range(32):
                nc.sync.dma_start(r8[:, kb * 4:(kb + 1) * 4], y7a_d[kb * 128:(kb + 1) * 128, :])

            ps8 = mmps.tile([128, 512], F32, tag="mm", name="mm")
            for kb in range(32):
                ws = smallp.tile([128, 64], F32, tag="w8s", name="w8s")
                nc.sync.dma_start(ws[:, :], w8t[kb * 128:(kb + 1) * 128, :])
                nc.tensor.matmul(ps8[0:64, 0:4], ws[:, :], r8[:, kb * 4:(kb + 1) * 4],
                                 start=(kb == 0), stop=(kb == 31))
            wb8t = smallp.tile([64, 1], F32, tag="wb8t", name="wb8t")
            nc.sync.dma_start(wb8t[:], wb8[:, :])
            y8 = smallp.tile([64, 4], F32, tag="y8", name="y8")
            nc.scalar.activation(y8[:], ps8[0:64, 0:4], AF.Tanh, bias=wb8t[:, 0:1])
            nc.sync.dma_start(y_out[:, :], y8[:])

    nc.compile()
    return nc


_NC_CACHE = {}


def kernel(**inputs):
    x = np.asarray(inputs["x"], dtype=np.float32)
    w = {k: np.asarray(v, dtype=np.float32) for k, v in inputs.items()}

    def pl(a, cols):
        a = np.asarray(a, np.float32).reshape(-1)
        out = np.zeros((128, cols), np.float32)
        out[np.arange(a.size) % 128, np.arange(a.size) // 128] = a
        return out

    base = {}
    for li, nm in enumerate(["1", "2", "3", "4"]):
        C, O = CS[li], OS[li]
        nt = max(1, O // 128)
        wl = w[f"w{nm}"]
        base[f"wdt{li}"] = np.ascontiguousarray(wl[:, :C].T)
        base[f"wvt{li}"] = np.ascontiguousarray((wl[:, C:] - wl[:, :C]).T)
        base[f"g{li}"] = pl(w[f"g{nm}"], nt)
        base[f"b{li}"] = pl(w[f"b{nm}"], nt)
    w5 = w["w5"]
    offs = [0, 3, 67, 195, 451, 963]
    for j in range(5):
        base[f"w5t{j}"] = np.ascontiguousarray(w5[:, offs[j]:offs[j + 1]].T)
    base["g5"] = pl(w["g5"], 16)
    base["b5"] = pl(w["b5"], 16)
    base["iota"] = np.tile(np.arange(N, dtype=np.int16), (128, 1))

    in_maps = []
    for c in range(8):
        b, h = c // 2, c % 2
        m = dict(base)
        xs = x[b]
        if h == 1:
            xs = np.concatenate([xs[:, NH:], xs[:, :NH]], axis=1)
        m["x"] = np.ascontiguousarray(xs)
        m["w6t"] = np.ascontiguousarray(w["w6"][c * 256:(c + 1) * 256, :].T)
        m["wb6"] = pl(w["wb6"][c * 256:(c + 1) * 256], 2)
        m["g6"] = pl(w["g6"][c * 256:(c + 1) * 256], 2)
        m["b6"] = pl(w["b6"][c * 256:(c + 1) * 256], 2)
        m["w7t"] = np.ascontiguousarray(w["w7"][c * 512:(c + 1) * 512, :].T)
        m["wb7"] = pl(w["wb7"][c * 512:(c + 1) * 512], 4)
        m["g7"] = pl(w["g7"][c * 512:(c + 1) * 512], 4)
        m["b7"] = pl(w["b7"][c * 512:(c + 1) * 512], 4)
        w8pad = np.zeros((64, 4096), np.float32)
        w8pad[:50] = w["w8"][c * 50:(c + 1) * 50, :]
        m["w8t"] = np.ascontiguousarray(w8pad.T)
        wb8pad = np.zeros((64, 1), np.float32)
        wb8pad[:50, 0] = w["wb8"][c * 50:(c + 1) * 50]
        m["wb8"] = wb8pad
        in_maps.append(m)

    key = ("k", DEBUG)
    if key not in _NC_CACHE:
        _NC_CACHE[key] = build(debug=DEBUG)
    nc = _NC_CACHE[key]
    res = run_bass_kernel_spmd(nc, in_maps, list(range(8)))
    kernel.last_results = res
    out = np.concatenate([res.results[c]["y_out"][:50, :] for c in range(8)], axis=0)
    return np.ascontiguousarray(out.T.astype(np.float32))


if __name__ == "__main__":
    _d = np.load("/root/problem/ref_data.npz")
    inputs = {k: _d[k] for k in _d.files if k != "expected"}
    out = kernel(**inputs)
    exp = _d["expected"]
    rel = np.abs(out - exp).max() / np.abs(exp).max()
    print("Relative error:", rel)
